# revision 28
# baseline (speedup 1.0000x reference)
"""GNN message-passing kernel for Trainium2 (8 NeuronCores, batch-parallel).

Computation (per reference):
    norm_adj = adjacency * dinv * dinv.T + I            [10,10]   (host, O(100) flops)
    support  = einsum('bcf,fo->bco', x, kernel)         [B,C,O]
    out      = elu(einsum('ij,bjo->bio', norm_adj, support) + bias)
    out      = (out - mean) * rsqrt(var+eps) * gamma + beta

Device strategy per core (512 batches = 5120 rows of [b,c] x f), all-bf16 PE:
  1. "Transposing mix": PE matmul with x-chunks [crows<=120, 128f] as the
     stationary operand and a block-diagonal norm_adj matrix [crows, crows]
     as the moving operand (bf16 runs full-rate at any moving width, unlike
     fp32r which needs >=256). One op applies the channel mix and lands the
     activations transposed ([f, rows]) as needed by the main matmul.
  2. Main matmul bf16: outT[o,rows] += K[f,o].T @ yT[f,rows], kernel matrix
     resident in SBUF as bf16 (FWL-eligible weight loads hide under matmuls).
     Panels of 480 rows (N=480 per matmul, one PSUM bank) amortize overheads.
  3. Epilogue on ACT/DVE with per-partition (o) params:
     elu(z) = min(exp(z), relu(z)+1) - 1 (exact), then folded BN affine.
     Output stored transposed [O, rows]; host transposes while unsharding.
"""

from contextlib import ExitStack

import numpy as np
import ml_dtypes

import concourse.bass as bass
import concourse.bacc as bacc
import concourse.mybir as mybir
import concourse.tile as tile
from concourse.bass_utils import run_bass_kernel_spmd

F32 = mybir.dt.float32
BF16 = mybir.dt.bfloat16
ALU = mybir.AluOpType
ACTF = mybir.ActivationFunctionType
NP_BF16 = ml_dtypes.bfloat16

P = 128
BN_EPS = 1e-3
N_CORES = 8
C = 10  # channels
BD_SIZES = (80, 120)  # distinct chunk row counts; bd blob has one 128-col slot each


def _panels_for(rows):
    a = rows // 480
    while a >= 0 and (rows - 480 * a) % 320 != 0:
        a -= 1
    assert a >= 0, f"rows={rows} not decomposable into 480/320 panels"
    b = (rows - 480 * a) // 320
    return [(12, 12, 12, 12)] * a + [(12, 12, 8)] * b


def build_nc(rows, F, O, n_cores=N_CORES, repeats=1,
             skip_mix=False, skip_main=False, skip_epi=False, force_320=False,
             mix_pair=False, out_bf16=False, store_split=False):
    """Build the per-core Bass program. rows = local (b,c) rows, F/O = feat dims.

    repeats>1 replays the whole computation (for timing amplification).
    skip_* flags are ablation hooks for performance attribution only.
    mix_pair: accumulate chunk pairs into one 240-wide PSUM group (fewer
    accumulation-group stops, stream-pipelined mix matmuls; 2x streamed cols)."""
    panels = [(12, 12, 8)] * (rows // 320) if force_320 else _panels_for(rows)
    maxpanel = max(sum(cb) * C for cb in panels)
    FT, OT = F // P, O // P

    nc = bacc.Bacc(
        "TRN2",
        target_bir_lowering=False,
        debug=False,
        enable_asserts=False,
        num_devices=n_cores,
    )
    x_d = nc.dram_tensor("x_local", [rows, F], BF16, kind="ExternalInput").ap()
    k_d = nc.dram_tensor("kern", [F, O], BF16, kind="ExternalInput").ap()
    # bd blob [P, 512]: cols 120:240 hold the 120-row block-diag (slice
    # [120:360] => [bd|0], [0:240] => [0|bd] for the pair trick); cols
    # 360:440 hold the 80-row block-diag; col block 440:512 unused.
    bd_d = nc.dram_tensor("bd", [P, 512], BF16, kind="ExternalInput").ap()
    # prm cols [0:OT]=bias_t, [OT:2OT]=scale_t, [2OT:3OT]=shift2_t (per-partition o)
    prm_d = nc.dram_tensor("prm", [P, 3 * OT], F32, kind="ExternalInput").ap()
    out_dt = BF16 if out_bf16 else F32
    outT_d = nc.dram_tensor("outT", [O, rows], out_dt, kind="ExternalOutput").ap()

    with tile.TileContext(nc) as tc, ExitStack() as ctx:
        const = ctx.enter_context(tc.tile_pool(name="const", bufs=1))
        bd = const.tile([P, 512], BF16, name="bd")
        prm = const.tile([P, 3 * OT], F32, name="prm")
        nc.sync.dma_start(bd, bd_d)
        nc.sync.dma_start(prm, prm_d)
        bd_t = {120: bd[:120, 120:240], 80: bd[:80, 360:440]}
        bd_pairA = bd[:120, 120:360]  # [bd120 | 0]
        bd_pairB = bd[:120, 0:240]    # [0 | bd120]
        kb = [const.tile([P, O], BF16, name=f"kb{fb}", tag=f"kb{fb}") for fb in range(FT)]
        for fb in range(FT):
            nc.scalar.dma_start(kb[fb], k_d[fb * P : (fb + 1) * P, :])

        xpool = ctx.enter_context(tc.tile_pool(name="xpool", bufs=8))
        ypool = ctx.enter_context(tc.tile_pool(name="ypool", bufs=2))
        mixps = ctx.enter_context(tc.tile_pool(name="mixps", bufs=3, space="PSUM"))
        mainps = ctx.enter_context(tc.tile_pool(name="mainps", bufs=5, space="PSUM"))
        tmp = ctx.enter_context(tc.tile_pool(name="tmp", bufs=3))

        ydummy = None
        if skip_mix:
            ydummy = const.tile([P, FT, maxpanel], BF16, name="ydummy")
            nc.gpsimd.memset(ydummy, 0.0)

        n_panels = len(panels)
        prow = [sum(cb) * C for cb in panels]
        rstart = [sum(prow[:i]) for i in range(n_panels)]
        # chunk offsets within each panel
        chunks = []  # per panel: list of (crows, coff)
        for cb in panels:
            offs, o = [], 0
            for nb in cb:
                offs.append((nb * C, o))
                o += nb * C
            chunks.append(offs)

        # mix unit descriptors per panel:
        #   ("pair", ciA, ciB, coffA, fbg)  -> [P, 2, 240] tile, 4 matmuls
        #   ("single", ci, crows, coff, fbp) -> [P, 4, crows] tile, 4 matmuls
        punits = []
        for pi, offs in enumerate(chunks):
            units = []
            if mix_pair:
                i = 0
                while i < len(offs):
                    if (i + 1 < len(offs) and offs[i][0] == 120
                            and offs[i + 1][0] == 120):
                        for fbg in range(FT // 2):
                            units.append(("pair", i, i + 1, offs[i][1], fbg))
                        i += 2
                    else:
                        for fbp in range(FT // 4):
                            units.append(("single", i, offs[i][0], offs[i][1], fbp))
                        i += 1
            else:
                for ci, (crows, coff) in enumerate(offs):
                    for fbp in range(FT // 4):
                        units.append(("single", ci, crows, coff, fbp))
            punits.append(units)

        seq = [(rep, pi) for rep in range(repeats) for pi in range(n_panels)]
        xt_tiles = {}
        yt_tiles = {}

        def emit_chunk_dma(rep, pi, ci):
            crows, coff = chunks[pi][ci]
            xt = xpool.tile([120, F], BF16, name=f"r{rep}_x_{pi}_{ci}", tag="xc")[:crows]
            nc.sync.dma_start(xt, x_d[rstart[pi] + coff : rstart[pi] + coff + crows, :])
            xt_tiles[(rep, pi, ci)] = xt

        def emit_mix_unit(rep, pi, u):
            unit = punits[pi][u]
            if (rep, pi) not in yt_tiles:
                yt_tiles[(rep, pi)] = ypool.tile(
                    [P, FT, maxpanel], BF16, name=f"r{rep}_yt_{pi}", tag="yt")
            ytall = yt_tiles[(rep, pi)]
            if unit[0] == "pair":
                _, ciA, ciB, coff, fbg = unit
                xtA = xt_tiles[(rep, pi, ciA)]
                xtB = xt_tiles[(rep, pi, ciB)]
                ps = mixps.tile([P, 2, 240], F32, name=f"r{rep}_mpp_{pi}_{u}", tag="mixps")
                for j in range(2):
                    fb = 2 * fbg + j
                    nc.tensor.matmul(
                        ps[:, j, :],
                        lhsT=xtA[:, fb * P : (fb + 1) * P],
                        rhs=bd_pairA,
                        start=True,
                        stop=False,
                    )
                    nc.tensor.matmul(
                        ps[:, j, :],
                        lhsT=xtB[:, fb * P : (fb + 1) * P],
                        rhs=bd_pairB,
                        start=False,
                        stop=True,
                    )
                nc.vector.tensor_copy(
                    ytall[:, 2 * fbg : 2 * fbg + 2, coff : coff + 240], ps
                )
            else:
                _, ci, crows, coff, fbp = unit
                xt = xt_tiles[(rep, pi, ci)]
                fb = 4 * fbp
                ps = mixps.tile([P, 4, 120], F32, name=f"r{rep}_mps_{pi}_{ci}_{fbp}", tag="mixps")
                for q in range(4):
                    nc.tensor.matmul(
                        ps[:, q, :crows],
                        lhsT=xt[:, (fb + q) * P : (fb + q + 1) * P],
                        rhs=bd_t[crows],
                        start=True,
                        stop=True,
                    )
                nc.vector.tensor_copy(
                    ytall[:, fb : fb + 4, coff : coff + crows], ps[:, :, :crows]
                )

        if skip_main:
            # mix-only ablation: sequential emission
            for rep, pi in seq:
                for ci in range(len(chunks[pi])):
                    emit_chunk_dma(rep, pi, ci)
                for u in range(len(punits[pi])):
                    emit_mix_unit(rep, pi, u)
        else:
            # software-pipelined: during main(s), prefetch x for s+2 and run
            # the mix (PE+copies) for s+1 interleaved between o-tile groups.
            if not skip_mix:
                for s0 in (0, 1):
                    if s0 < len(seq):
                        for ci in range(len(chunks[seq[s0][1]])):
                            emit_chunk_dma(seq[s0][0], seq[s0][1], ci)
                for u in range(len(punits[seq[0][1]])):
                    emit_mix_unit(seq[0][0], seq[0][1], u)

            for s, (rep, pi) in enumerate(seq):
                nxt = seq[s + 1] if s + 1 < len(seq) else None
                nxt2 = seq[s + 2] if s + 2 < len(seq) else None
                panel = prow[pi]
                row0 = rstart[pi]
                ytall = ydummy if skip_mix else yt_tiles[(rep, pi)]
                n_units_next = len(punits[nxt[1]]) if nxt else 0
                for ot in range(OT):
                    if not skip_mix:
                        if ot == 0 and nxt2 is not None:
                            for ci in range(len(chunks[nxt2[1]])):
                                emit_chunk_dma(nxt2[0], nxt2[1], ci)
                        if nxt is not None and ot < n_units_next:
                            emit_mix_unit(nxt[0], nxt[1], ot)
                    ps = mainps.tile([P, maxpanel], F32, name=f"r{rep}_ops_{pi}_{ot}", tag="mainps")[:, :panel]
                    for fb in range(FT):
                        nc.tensor.matmul(
                            ps,
                            lhsT=kb[fb][:, ot * P : (ot + 1) * P],
                            rhs=ytall[:, fb, :panel],
                            start=(fb == 0),
                            stop=(fb == FT - 1),
                        )
                    if skip_epi:
                        continue
                    bias_ap = prm[:, ot : ot + 1]
                    scale_ap = prm[:, OT + ot : OT + ot + 1]
                    shift_ap = prm[:, 2 * OT + ot : 2 * OT + ot + 1]
                    e = tmp.tile([P, maxpanel], BF16, name=f"r{rep}_e_{pi}_{ot}", tag="e")[:, :panel]
                    t0 = tmp.tile([P, maxpanel], BF16, name=f"r{rep}_t0_{pi}_{ot}", tag="t0")[:, :panel]
                    s_ = tmp.tile([P, maxpanel], BF16, name=f"r{rep}_s_{pi}_{ot}", tag="s")[:, :panel]
                    fin = tmp.tile([P, maxpanel], out_dt, name=f"r{rep}_fin_{pi}_{ot}", tag="fin")[:, :panel]
                    nc.scalar.activation(e, ps, ACTF.Exp, bias=bias_ap)
                    nc.scalar.activation(t0, ps, ACTF.Relu, bias=bias_ap)
                    # elu(zb) + 1 = min(exp(zb), relu(zb) + 1)   (exact identity)
                    nc.vector.scalar_tensor_tensor(
                        s_, in0=t0, scalar=1.0, in1=e, op0=ALU.add, op1=ALU.min
                    )
                    # fin = s*scale + (shift - scale) = elu*scale + shift
                    nc.vector.tensor_scalar(
                        fin, s_, scale_ap, shift_ap, op0=ALU.mult, op1=ALU.add
                    )
                    st_eng = nc.sync if (store_split and ot % 2) else nc.scalar
                    st_eng.dma_start(outT_d[ot * P : (ot + 1) * P, row0 : row0 + panel], fin)
    nc.compile()
    return nc


def _host_prep(adjacency, kern, bias, gamma, beta, moving_mean, moving_var, O=2048):
    """Build the tiny derived inputs on the host. Returns (bd, prm) arrays."""
    A = np.asarray(adjacency, np.float32)
    deg = np.maximum(np.abs(A).sum(axis=1, keepdims=True), 1e-8)
    dinv = deg ** -0.5
    na = A * dinv * dinv.T + np.eye(C, dtype=np.float32)  # [10,10]

    OT = O // P
    # [P, 512]: cols 120:240 = 120-row block diag (so [120:360] reads
    # [bd|0] and [0:240] reads [0|bd]); cols 360:440 = 80-row block diag.
    bd = np.zeros((P, 512), NP_BF16)
    naT = na.T.astype(NP_BF16)
    for g in range(120 // C):
        bd[g * C : (g + 1) * C, 120 + g * C : 120 + (g + 1) * C] = naT
    for g in range(80 // C):
        bd[g * C : (g + 1) * C, 360 + g * C : 360 + (g + 1) * C] = naT
    scale = np.asarray(gamma, np.float32) / np.sqrt(np.asarray(moving_var, np.float32) + BN_EPS)
    shift2 = np.asarray(beta, np.float32) - np.asarray(moving_mean, np.float32) * scale - scale
    prm = np.zeros((P, 3 * OT), np.float32)
    prm[:, 0:OT] = np.asarray(bias, np.float32).reshape(OT, P).T
    prm[:, OT : 2 * OT] = scale.reshape(OT, P).T
    prm[:, 2 * OT : 3 * OT] = shift2.reshape(OT, P).T
    return bd, prm


def prepare_in_maps(x, adjacency, kernel, bias, gamma, beta, moving_mean, moving_var):
    """Shard + cast all host inputs. Returns (in_maps, rows)."""
    B, C_, F = x.shape
    O = kernel.shape[1]
    assert C_ == C
    assert B % N_CORES == 0
    bl = B // N_CORES
    rows = bl * C

    bd, prm = _host_prep(adjacency, kernel, bias, gamma, beta, moving_mean,
                         moving_var, O)
    kern_bf = np.ascontiguousarray(np.asarray(kernel, np.float32).astype(NP_BF16))
    x_bf = np.asarray(x, np.float32).astype(NP_BF16)
    in_maps = []
    for c in range(N_CORES):
        in_maps.append({
            "x_local": np.ascontiguousarray(x_bf[c * bl : (c + 1) * bl].reshape(rows, F)),
            "kern": kern_bf,
            "bd": bd,
            "prm": prm,
        })
    return in_maps, rows


def kernel(x, adjacency, kernel, bias, gamma, beta, moving_mean, moving_var):
    B, C_, F = x.shape
    O = kernel.shape[1]
    bl = B // N_CORES
    in_maps, rows = prepare_in_maps(x, adjacency, kernel, bias, gamma, beta,
                                    moving_mean, moving_var)
    nc = build_nc(rows, F, O)
    res = run_bass_kernel_spmd(nc, in_maps, core_ids=list(range(N_CORES)), trace=False)

    out = np.empty((B, C, O), np.float32)
    for c in range(N_CORES):
        outT = np.asarray(res.results[c]["outT"], np.float32)  # [O, rows]
        out[c * bl : (c + 1) * bl] = outT.T.reshape(bl, C, O)
    return out


# revision 29
# speedup vs baseline: 1.1263x; 1.1263x over previous
"""GNN message-passing kernel for Trainium2 (8 NeuronCores, batch-parallel).

Computation (per reference):
    norm_adj = adjacency * dinv * dinv.T + I            [10,10]   (host, O(100) flops)
    support  = einsum('bcf,fo->bco', x, kernel)         [B,C,O]
    out      = elu(einsum('ij,bjo->bio', norm_adj, support) + bias)
    out      = (out - mean) * rsqrt(var+eps) * gamma + beta

Device strategy per core (512 batches = 5120 rows of [b,c] x f), all-bf16 PE:
  1. "Transposing mix": PE matmul with x-chunks [crows<=120, 128f] as the
     stationary operand and a block-diagonal norm_adj matrix [crows, crows]
     as the moving operand (bf16 runs full-rate at any moving width, unlike
     fp32r which needs >=256). One op applies the channel mix and lands the
     activations transposed ([f, rows]) as needed by the main matmul.
  2. Main matmul bf16: outT[o,rows] += K[f,o].T @ yT[f,rows], kernel matrix
     resident in SBUF as bf16 (FWL-eligible weight loads hide under matmuls).
     Panels of 480 rows (N=480 per matmul, one PSUM bank) amortize overheads.
  3. Software pipelining: panel p+1's mix units (4 matmuls + 1 PSUM->SBUF
     copy each) are emitted interleaved between panel p's main o-tile
     groups, and panel p+2's x DMAs prefetch at p's start, so the PE never
     waits for mix copies at panel boundaries (PE ~95% occupied in sim).
  4. Epilogue on ACT/DVE with per-partition (o) params:
     elu(z) = min(exp(z), relu(z)+1) - 1 (exact), then folded BN affine.
     Output stored transposed [O, rows] via the ACT HWDGE queue (input
     loads go via the SP queue - separate rings); host transposes while
     unsharding.
"""

from contextlib import ExitStack

import numpy as np
import ml_dtypes

import concourse.bass as bass
import concourse.bacc as bacc
import concourse.mybir as mybir
import concourse.tile as tile
from concourse.bass_utils import run_bass_kernel_spmd

F32 = mybir.dt.float32
BF16 = mybir.dt.bfloat16
ALU = mybir.AluOpType
ACTF = mybir.ActivationFunctionType
NP_BF16 = ml_dtypes.bfloat16

P = 128
BN_EPS = 1e-3
N_CORES = 8
C = 10  # channels
BD_SIZES = (80, 120)  # distinct chunk row counts; bd blob has one 128-col slot each


def _panels_for(rows):
    a = rows // 480
    while a >= 0 and (rows - 480 * a) % 320 != 0:
        a -= 1
    assert a >= 0, f"rows={rows} not decomposable into 480/320 panels"
    b = (rows - 480 * a) // 320
    return [(12, 12, 12, 12)] * a + [(12, 12, 8)] * b


def build_nc(rows, F, O, n_cores=N_CORES, repeats=1,
             skip_mix=False, skip_main=False, skip_epi=False, force_320=False,
             mix_pair=False, out_bf16=False, store_split=False):
    """Build the per-core Bass program. rows = local (b,c) rows, F/O = feat dims.

    repeats>1 replays the whole computation (for timing amplification).
    skip_* flags are ablation hooks for performance attribution only.
    mix_pair: accumulate chunk pairs into one 240-wide PSUM group (fewer
    accumulation-group stops, stream-pipelined mix matmuls; 2x streamed cols)."""
    panels = [(12, 12, 8)] * (rows // 320) if force_320 else _panels_for(rows)
    maxpanel = max(sum(cb) * C for cb in panels)
    FT, OT = F // P, O // P

    nc = bacc.Bacc(
        "TRN2",
        target_bir_lowering=False,
        debug=False,
        enable_asserts=False,
        num_devices=n_cores,
    )
    x_d = nc.dram_tensor("x_local", [rows, F], BF16, kind="ExternalInput").ap()
    k_d = nc.dram_tensor("kern", [F, O], BF16, kind="ExternalInput").ap()
    # bd blob [P, 512]: cols 120:240 hold the 120-row block-diag (slice
    # [120:360] => [bd|0], [0:240] => [0|bd] for the pair trick); cols
    # 360:440 hold the 80-row block-diag; col block 440:512 unused.
    bd_d = nc.dram_tensor("bd", [P, 512], BF16, kind="ExternalInput").ap()
    # prm cols [0:OT]=bias_t, [OT:2OT]=scale_t, [2OT:3OT]=shift2_t (per-partition o)
    prm_d = nc.dram_tensor("prm", [P, 3 * OT], F32, kind="ExternalInput").ap()
    out_dt = BF16 if out_bf16 else F32
    outT_d = nc.dram_tensor("outT", [O, rows], out_dt, kind="ExternalOutput").ap()

    with tile.TileContext(nc) as tc, ExitStack() as ctx:
        const = ctx.enter_context(tc.tile_pool(name="const", bufs=1))
        bd = const.tile([P, 512], BF16, name="bd")
        prm = const.tile([P, 3 * OT], F32, name="prm")
        nc.sync.dma_start(bd, bd_d)
        nc.sync.dma_start(prm, prm_d)
        bd_t = {120: bd[:120, 120:240], 80: bd[:80, 360:440]}
        bd_pairA = bd[:120, 120:360]  # [bd120 | 0]
        bd_pairB = bd[:120, 0:240]    # [0 | bd120]
        kb = [const.tile([P, O], BF16, name=f"kb{fb}", tag=f"kb{fb}") for fb in range(FT)]
        for fb in range(FT):
            nc.scalar.dma_start(kb[fb], k_d[fb * P : (fb + 1) * P, :])

        xpool = ctx.enter_context(tc.tile_pool(name="xpool", bufs=8))
        ypool = ctx.enter_context(tc.tile_pool(name="ypool", bufs=2))
        mixps = ctx.enter_context(tc.tile_pool(name="mixps", bufs=3, space="PSUM"))
        mainps = ctx.enter_context(tc.tile_pool(name="mainps", bufs=5, space="PSUM"))
        tmp = ctx.enter_context(tc.tile_pool(name="tmp", bufs=3))

        ydummy = None
        if skip_mix:
            ydummy = const.tile([P, FT, maxpanel], BF16, name="ydummy")
            nc.gpsimd.memset(ydummy, 0.0)

        n_panels = len(panels)
        prow = [sum(cb) * C for cb in panels]
        rstart = [sum(prow[:i]) for i in range(n_panels)]
        # chunk offsets within each panel
        chunks = []  # per panel: list of (crows, coff)
        for cb in panels:
            offs, o = [], 0
            for nb in cb:
                offs.append((nb * C, o))
                o += nb * C
            chunks.append(offs)

        # mix unit descriptors per panel:
        #   ("pair", ciA, ciB, coffA, fbg)  -> [P, 2, 240] tile, 4 matmuls
        #   ("single", ci, crows, coff, fbp) -> [P, 4, crows] tile, 4 matmuls
        punits = []
        for pi, offs in enumerate(chunks):
            units = []
            if mix_pair:
                i = 0
                while i < len(offs):
                    if (i + 1 < len(offs) and offs[i][0] == 120
                            and offs[i + 1][0] == 120):
                        for fbg in range(FT // 2):
                            units.append(("pair", i, i + 1, offs[i][1], fbg))
                        i += 2
                    else:
                        for fbp in range(FT // 4):
                            units.append(("single", i, offs[i][0], offs[i][1], fbp))
                        i += 1
            else:
                for ci, (crows, coff) in enumerate(offs):
                    for fbp in range(FT // 4):
                        units.append(("single", ci, crows, coff, fbp))
            punits.append(units)

        seq = [(rep, pi) for rep in range(repeats) for pi in range(n_panels)]
        xt_tiles = {}
        yt_tiles = {}

        def emit_chunk_dma(rep, pi, ci):
            crows, coff = chunks[pi][ci]
            xt = xpool.tile([120, F], BF16, name=f"r{rep}_x_{pi}_{ci}", tag="xc")[:crows]
            nc.sync.dma_start(xt, x_d[rstart[pi] + coff : rstart[pi] + coff + crows, :])
            xt_tiles[(rep, pi, ci)] = xt

        def emit_mix_unit(rep, pi, u):
            unit = punits[pi][u]
            if (rep, pi) not in yt_tiles:
                yt_tiles[(rep, pi)] = ypool.tile(
                    [P, FT, maxpanel], BF16, name=f"r{rep}_yt_{pi}", tag="yt")
            ytall = yt_tiles[(rep, pi)]
            if unit[0] == "pair":
                _, ciA, ciB, coff, fbg = unit
                xtA = xt_tiles[(rep, pi, ciA)]
                xtB = xt_tiles[(rep, pi, ciB)]
                ps = mixps.tile([P, 2, 240], F32, name=f"r{rep}_mpp_{pi}_{u}", tag="mixps")
                for j in range(2):
                    fb = 2 * fbg + j
                    nc.tensor.matmul(
                        ps[:, j, :],
                        lhsT=xtA[:, fb * P : (fb + 1) * P],
                        rhs=bd_pairA,
                        start=True,
                        stop=False,
                    )
                    nc.tensor.matmul(
                        ps[:, j, :],
                        lhsT=xtB[:, fb * P : (fb + 1) * P],
                        rhs=bd_pairB,
                        start=False,
                        stop=True,
                    )
                nc.vector.tensor_copy(
                    ytall[:, 2 * fbg : 2 * fbg + 2, coff : coff + 240], ps
                )
            else:
                _, ci, crows, coff, fbp = unit
                xt = xt_tiles[(rep, pi, ci)]
                fb = 4 * fbp
                ps = mixps.tile([P, 4, 120], F32, name=f"r{rep}_mps_{pi}_{ci}_{fbp}", tag="mixps")
                for q in range(4):
                    nc.tensor.matmul(
                        ps[:, q, :crows],
                        lhsT=xt[:, (fb + q) * P : (fb + q + 1) * P],
                        rhs=bd_t[crows],
                        start=True,
                        stop=True,
                    )
                nc.vector.tensor_copy(
                    ytall[:, fb : fb + 4, coff : coff + crows], ps[:, :, :crows]
                )

        if skip_main:
            # mix-only ablation: sequential emission
            for rep, pi in seq:
                for ci in range(len(chunks[pi])):
                    emit_chunk_dma(rep, pi, ci)
                for u in range(len(punits[pi])):
                    emit_mix_unit(rep, pi, u)
        else:
            # software-pipelined: during main(s), prefetch x for s+2 and run
            # the mix (PE+copies) for s+1 interleaved between o-tile groups.
            if not skip_mix:
                for s0 in (0, 1):
                    if s0 < len(seq):
                        for ci in range(len(chunks[seq[s0][1]])):
                            emit_chunk_dma(seq[s0][0], seq[s0][1], ci)
                for u in range(len(punits[seq[0][1]])):
                    emit_mix_unit(seq[0][0], seq[0][1], u)

            for s, (rep, pi) in enumerate(seq):
                nxt = seq[s + 1] if s + 1 < len(seq) else None
                nxt2 = seq[s + 2] if s + 2 < len(seq) else None
                panel = prow[pi]
                row0 = rstart[pi]
                ytall = ydummy if skip_mix else yt_tiles[(rep, pi)]
                n_units_next = len(punits[nxt[1]]) if nxt else 0
                for ot in range(OT):
                    if not skip_mix:
                        if ot == 0 and nxt2 is not None:
                            for ci in range(len(chunks[nxt2[1]])):
                                emit_chunk_dma(nxt2[0], nxt2[1], ci)
                        if nxt is not None and ot < n_units_next:
                            emit_mix_unit(nxt[0], nxt[1], ot)
                    ps = mainps.tile([P, maxpanel], F32, name=f"r{rep}_ops_{pi}_{ot}", tag="mainps")[:, :panel]
                    for fb in range(FT):
                        nc.tensor.matmul(
                            ps,
                            lhsT=kb[fb][:, ot * P : (ot + 1) * P],
                            rhs=ytall[:, fb, :panel],
                            start=(fb == 0),
                            stop=(fb == FT - 1),
                        )
                    if skip_epi:
                        continue
                    bias_ap = prm[:, ot : ot + 1]
                    scale_ap = prm[:, OT + ot : OT + ot + 1]
                    shift_ap = prm[:, 2 * OT + ot : 2 * OT + ot + 1]
                    e = tmp.tile([P, maxpanel], BF16, name=f"r{rep}_e_{pi}_{ot}", tag="e")[:, :panel]
                    t0 = tmp.tile([P, maxpanel], BF16, name=f"r{rep}_t0_{pi}_{ot}", tag="t0")[:, :panel]
                    s_ = tmp.tile([P, maxpanel], BF16, name=f"r{rep}_s_{pi}_{ot}", tag="s")[:, :panel]
                    fin = tmp.tile([P, maxpanel], out_dt, name=f"r{rep}_fin_{pi}_{ot}", tag="fin")[:, :panel]
                    nc.scalar.activation(e, ps, ACTF.Exp, bias=bias_ap)
                    nc.scalar.activation(t0, ps, ACTF.Relu, bias=bias_ap)
                    # elu(zb) + 1 = min(exp(zb), relu(zb) + 1)   (exact identity)
                    nc.vector.scalar_tensor_tensor(
                        s_, in0=t0, scalar=1.0, in1=e, op0=ALU.add, op1=ALU.min
                    )
                    # fin = s*scale + (shift - scale) = elu*scale + shift
                    nc.vector.tensor_scalar(
                        fin, s_, scale_ap, shift_ap, op0=ALU.mult, op1=ALU.add
                    )
                    st_eng = nc.sync if (store_split and ot % 2) else nc.scalar
                    st_eng.dma_start(outT_d[ot * P : (ot + 1) * P, row0 : row0 + panel], fin)
    nc.compile()
    return nc


def _host_prep(adjacency, kern, bias, gamma, beta, moving_mean, moving_var, O=2048):
    """Build the tiny derived inputs on the host. Returns (bd, prm) arrays."""
    A = np.asarray(adjacency, np.float32)
    deg = np.maximum(np.abs(A).sum(axis=1, keepdims=True), 1e-8)
    dinv = deg ** -0.5
    na = A * dinv * dinv.T + np.eye(C, dtype=np.float32)  # [10,10]

    OT = O // P
    # [P, 512]: cols 120:240 = 120-row block diag (so [120:360] reads
    # [bd|0] and [0:240] reads [0|bd]); cols 360:440 = 80-row block diag.
    bd = np.zeros((P, 512), NP_BF16)
    naT = na.T.astype(NP_BF16)
    for g in range(120 // C):
        bd[g * C : (g + 1) * C, 120 + g * C : 120 + (g + 1) * C] = naT
    for g in range(80 // C):
        bd[g * C : (g + 1) * C, 360 + g * C : 360 + (g + 1) * C] = naT
    scale = np.asarray(gamma, np.float32) / np.sqrt(np.asarray(moving_var, np.float32) + BN_EPS)
    shift2 = np.asarray(beta, np.float32) - np.asarray(moving_mean, np.float32) * scale - scale
    prm = np.zeros((P, 3 * OT), np.float32)
    prm[:, 0:OT] = np.asarray(bias, np.float32).reshape(OT, P).T
    prm[:, OT : 2 * OT] = scale.reshape(OT, P).T
    prm[:, 2 * OT : 3 * OT] = shift2.reshape(OT, P).T
    return bd, prm


def prepare_in_maps(x, adjacency, kernel, bias, gamma, beta, moving_mean, moving_var):
    """Shard + cast all host inputs. Returns (in_maps, rows)."""
    B, C_, F = x.shape
    O = kernel.shape[1]
    assert C_ == C
    assert B % N_CORES == 0
    bl = B // N_CORES
    rows = bl * C

    bd, prm = _host_prep(adjacency, kernel, bias, gamma, beta, moving_mean,
                         moving_var, O)
    kern_bf = np.ascontiguousarray(np.asarray(kernel, np.float32).astype(NP_BF16))
    x_bf = np.asarray(x, np.float32).astype(NP_BF16)
    in_maps = []
    for c in range(N_CORES):
        in_maps.append({
            "x_local": np.ascontiguousarray(x_bf[c * bl : (c + 1) * bl].reshape(rows, F)),
            "kern": kern_bf,
            "bd": bd,
            "prm": prm,
        })
    return in_maps, rows


def kernel(x, adjacency, kernel, bias, gamma, beta, moving_mean, moving_var):
    B, C_, F = x.shape
    O = kernel.shape[1]
    bl = B // N_CORES
    in_maps, rows = prepare_in_maps(x, adjacency, kernel, bias, gamma, beta,
                                    moving_mean, moving_var)
    nc = build_nc(rows, F, O)
    res = run_bass_kernel_spmd(nc, in_maps, core_ids=list(range(N_CORES)), trace=False)

    out = np.empty((B, C, O), np.float32)
    for c in range(N_CORES):
        outT = np.asarray(res.results[c]["outT"], np.float32)  # [O, rows]
        out[c * bl : (c + 1) * bl] = outT.T.reshape(bl, C, O)
    return out
